# revision 5
# baseline (speedup 1.0000x reference)
"""DiffPathRenderer Trainium2 kernel.

Renders an anti-aliased stroke (darkness = clip((r - dist)/r, 0, 1), where
dist is per-pixel min distance to a 63-segment polyline) on a 512x512 canvas,
sharded 64 rows per core across 8 NeuronCores.

Math: for segment k with start v, direction s, squared length d2, define
per pixel p:
    dot   = (p - v) . s
    ahat  = (dot - d2/2) / sqrt(d2)          # axial offset from segment center
    b     = sqrt(d2) / 2                     # half segment length
    axial = relu(|ahat| - b)                 # distance beyond segment ends
    perp2 = |p - v|^2 - dot^2 / d2           # squared perpendicular distance
    dist2 = max(perp2, 0) + axial^2
ahat and perp2 are quadratic polynomials in the pixel x-coordinate when the
pixel block lies within one image row, so a single TensorE matmul with a
constant stationary feature matrix [px'^2, px', 1] (px' = x - block_center)
produces both for all 63 segments at once.  Per-block coefficient matrices
are precomputed on host (O(n_blocks * n_segs) scalar work).  The per-block
local origin keeps all matmul terms small so fp32 accumulation keeps ~1e-3
absolute precision on dist2 near the stroke band.
"""

import numpy as np

import concourse.bacc as bacc
import concourse.mybir as mybir
import concourse.tile as tile
from concourse.bass_utils import run_bass_kernel_spmd

F32 = mybir.dt.float32
S = 512
NPTS = 64
NSEG = NPTS - 1           # 63
NCORES = 8
ROWS_PER_CORE = S // NCORES   # 64
BLK = 128                 # pixels per block (quarter of an image row)
BPR = S // BLK            # 4 blocks per row
NBLK = ROWS_PER_CORE * BPR    # 256 blocks per core
WCOLS = 2 * NSEG          # 126 coefficient columns per block (ahat | perp2)
WAVE = 16                 # blocks per wave: 16*128 padded cols = 4 PSUM banks
NWAVE = NBLK // WAVE      # 16


def _build_kernel(radius: float):
    nc = bacc.Bacc(
        "TRN2", target_bir_lowering=False, debug=False, num_devices=NCORES
    )

    f0_d = nc.dram_tensor("f0", [3, BLK], F32, kind="ExternalInput")
    w_d = nc.dram_tensor("w", [3, NBLK * WCOLS], F32, kind="ExternalInput")
    ctb_d = nc.dram_tensor("ctb", [128, WAVE * NSEG], F32, kind="ExternalInput")
    ident_d = nc.dram_tensor("ident", [128, 128], F32, kind="ExternalInput")
    out_d = nc.dram_tensor("out", [2, 128, 128], F32, kind="ExternalOutput")

    with tile.TileContext(nc) as tc:
        with (
            tc.tile_pool(name="const", bufs=1) as cpool,
            tc.tile_pool(name="acc", bufs=1) as apool,
            tc.tile_pool(name="wstream", bufs=2) as wpool,
            tc.tile_pool(name="work", bufs=2) as wk,
        ):
            f0 = cpool.tile([3, BLK], F32)
            nc.sync.dma_start(f0[:], f0_d[:])
            ctb = cpool.tile([128, WAVE * NSEG], F32)
            nc.sync.dma_start(ctb[:], ctb_d[:])
            ident = cpool.tile([128, 128], F32)
            nc.sync.dma_start(ident[:], ident_d[:])

            macc = apool.tile([128, NBLK], F32)

            ctb3 = ctb[:].rearrange("p (w s) -> p w s", w=WAVE)

            with tc.tile_pool(name="psum_mm", bufs=2, space="PSUM") as pp:
                for wv in range(NWAVE):
                    wsb = wpool.tile([3, WAVE * WCOLS], F32)
                    nc.sync.dma_start(
                        wsb[:],
                        w_d[:, wv * WAVE * WCOLS : (wv + 1) * WAVE * WCOLS],
                    )
                    ps = pp.tile([128, WAVE, 128], F32)
                    for b in range(WAVE):
                        nc.tensor.matmul(
                            ps[:, b, 0:WCOLS],
                            lhsT=f0[:, :],
                            rhs=wsb[:, b * WCOLS : (b + 1) * WCOLS],
                        )
                    ahat = ps[:, :, 0:NSEG]
                    perp2 = ps[:, :, NSEG:WCOLS]

                    ab = wk.tile([128, WAVE, NSEG], F32, tag="ab")
                    nc.scalar.activation(
                        ab[:], ahat, mybir.ActivationFunctionType.Abs
                    )
                    # u = relu(|ahat| - b): (ab max 0.0) sub ctb, then Relu
                    u = wk.tile([128, WAVE, NSEG], F32, tag="u")
                    nc.vector.scalar_tensor_tensor(
                        u[:], ab[:], 0.0, ctb3,
                        op0=mybir.AluOpType.max,
                        op1=mybir.AluOpType.subtract,
                    )
                    r = wk.tile([128, WAVE, NSEG], F32, tag="r")
                    nc.scalar.activation(
                        r[:], u[:], mybir.ActivationFunctionType.Relu
                    )
                    z = wk.tile([128, WAVE, NSEG], F32, tag="z")
                    nc.scalar.activation(
                        z[:], r[:], mybir.ActivationFunctionType.Square
                    )
                    d2t = wk.tile([128, WAVE, NSEG], F32, tag="d2t")
                    nc.vector.scalar_tensor_tensor(
                        d2t[:], perp2, 0.0, z[:],
                        op0=mybir.AluOpType.max,
                        op1=mybir.AluOpType.add,
                    )
                    nc.vector.tensor_reduce(
                        macc[:, wv * WAVE : (wv + 1) * WAVE],
                        d2t[:],
                        axis=mybir.AxisListType.X,
                        op=mybir.AluOpType.min,
                    )

            with tc.tile_pool(name="psum_t", bufs=2, space="PSUM") as pt:
                for h in range(2):
                    tp = pt.tile([128, 128], F32)
                    nc.tensor.matmul(
                        tp[:],
                        lhsT=macc[:, h * 128 : (h + 1) * 128],
                        rhs=ident[:],
                        is_transpose=True,
                    )
                    sq = wk.tile([128, 128], F32, tag="sq")
                    nc.scalar.activation(
                        sq[:], tp[:], mybir.ActivationFunctionType.Sqrt
                    )
                    o = wk.tile([128, 128], F32, tag="o")
                    nc.scalar.activation(
                        o[:], sq[:], mybir.ActivationFunctionType.Relu,
                        scale=-1.0 / radius, bias=1.0,
                    )
                    nc.sync.dma_start(out_d[h], o[:])

    nc.compile()
    return nc


def _host_coeffs(traj: np.ndarray):
    """Per-core W matrices [3, NBLK*126] plus shared consts."""
    t = traj.astype(np.float64) * S
    v = t[:-1]
    seg = t[1:] - v
    sx, sy = seg[:, 0], seg[:, 1]
    d2 = sx * sx + sy * sy
    sq = np.sqrt(d2)

    mvals = np.arange(BLK, dtype=np.float64) - 64.0
    f0 = np.stack([mvals * mvals, mvals, np.ones(BLK)]).astype(np.float32)

    ctb_row = (sq / 2).astype(np.float32)
    ctb = np.tile(ctb_row[None, :], (128, WAVE)).astype(np.float32)

    ws = []
    for core in range(NCORES):
        W = np.zeros((NBLK, 3, WCOLS), np.float64)
        for blk in range(NBLK):
            oy = core * ROWS_PER_CORE + blk // BPR
            ox = (blk % BPR) * BLK + 64.0
            rx = ox - v[:, 0]
            ry = oy - v[:, 1]
            m0 = rx * sx + ry * sy
            W[blk, 0, 0:NSEG] = 0.0
            W[blk, 1, 0:NSEG] = sx / sq
            W[blk, 2, 0:NSEG] = (m0 - d2 / 2) / sq
            W[blk, 0, NSEG:] = 1.0 - sx * sx / d2
            W[blk, 1, NSEG:] = 2 * rx - 2 * sx * m0 / d2
            W[blk, 2, NSEG:] = rx * rx + ry * ry - m0 * m0 / d2
        ws.append(
            np.ascontiguousarray(W.transpose(1, 0, 2).reshape(3, NBLK * WCOLS))
            .astype(np.float32)
        )
    return f0, ws, ctb


def build_for_sim(np_inputs):
    """Build the (un-run) Bass module for cost-model simulation in test.py."""
    return _build_kernel(float(np_inputs["thickness"]) / 2.0)


def kernel(traj: np.ndarray, thickness: np.ndarray) -> np.ndarray:
    radius = float(np.asarray(thickness)) / 2.0
    f0, ws, ctb = _host_coeffs(np.asarray(traj, np.float32))
    ident = np.eye(128, dtype=np.float32)

    nc = _build_kernel(radius)
    in_maps = [
        {"f0": f0, "w": ws[c], "ctb": ctb, "ident": ident}
        for c in range(NCORES)
    ]
    res = run_bass_kernel_spmd(nc, in_maps, core_ids=list(range(NCORES)))

    out = np.empty((S, S), np.float32)
    for c in range(NCORES):
        slab = res.results[c]["out"].reshape(ROWS_PER_CORE, S)
        out[c * ROWS_PER_CORE : (c + 1) * ROWS_PER_CORE] = slab
    return out
